# revision 1
# baseline (speedup 1.0000x reference)
"""InfoNCE loss kernel for Trainium2, 8 NeuronCores.

Reference computation:
    z = l2_normalize(concat([polyline_embs, c_embs]))   # [8192, 512]
    sim = z @ z.T                                       # [8192, 8192]
    denom_i = sum_{j != i} exp(sim_ij / T)
    pos_i   = sim[i, i +- B]
    loss    = mean(log(denom_i) - pos_i / T)

Sharding: row-block parallel. Core k computes rows [k*1024, (k+1)*1024) of sim
against all 8192 columns (the "all-gather" is free: every core receives the
full transposed embedding matrix as input). The scalar loss is reduced on host
(the "all-reduce").

Each core runs an identical program on different input slices (SPMD without
partition-id): per-core inputs are pre-sliced on host.
"""

import os

import numpy as np
import ml_dtypes

B = 4096
D = 512
N = 2 * B            # 8192 rows of sim
NCORES = 8
RPC = N // NCORES    # 1024 rows per core
P = 128              # partitions
ITILES = RPC // P    # 8 row tiles per core
CTILES = D // P      # 4 contraction chunks of 128
NT = 512             # column-tile width
NTILES = N // NT     # 16 column tiles
INV_T = 2.0          # 1 / temperature

_CACHE = {}


def _build_bass():
    """Trace the per-core Bass program (identical for all 8 cores)."""
    import concourse.bass as bass
    import concourse.tile as tile
    from concourse import bacc, mybir

    dt = mybir.dt
    AF = mybir.ActivationFunctionType
    ALU = mybir.AluOpType

    nc = bacc.Bacc(None, target_bir_lowering=False, debug=False, num_swdge_queues=4)

    # -------- DRAM I/O (host-pretiled, bf16) --------
    # xa: full Z^T, tiled per column-tile: [n][p][c][col], element = x[c*128+p, n*512+col]
    xa_d = nc.dram_tensor("xa", [NTILES, P, CTILES, NT], dt.bfloat16, kind="ExternalInput")
    # xm: my row block, transposed: [p][c][1024], element = x_mine[c*128+p, row]
    xm_d = nc.dram_tensor("xm", [P, CTILES, RPC], dt.bfloat16, kind="ExternalInput")
    # xmT/xpT: my / partner row blocks, row-major tiled: [p][i][d]
    xmT_d = nc.dram_tensor("xmT", [P, ITILES, D], dt.bfloat16, kind="ExternalInput")
    xpT_d = nc.dram_tensor("xpT", [P, ITILES, D], dt.bfloat16, kind="ExternalInput")

    out_d = nc.dram_tensor("loss_rows", [P, ITILES], dt.float32, kind="ExternalOutput")
    dbg_d = nc.dram_tensor("dbg", [P, ITILES, 4], dt.float32, kind="ExternalOutput")

    from contextlib import ExitStack

    with tile.TileContext(nc) as tc, ExitStack() as ctx:
        const = ctx.enter_context(tc.tile_pool(name="const", bufs=1))
        prol = ctx.enter_context(tc.tile_pool(name="prol", bufs=1))
        persist = ctx.enter_context(tc.tile_pool(name="persist", bufs=1))
        nstream = ctx.enter_context(tc.tile_pool(name="nstream", bufs=3))
        small = ctx.enter_context(tc.tile_pool(name="small", bufs=3))
        junkp = ctx.enter_context(tc.tile_pool(name="junk", bufs=4))
        psum_s = ctx.enter_context(tc.tile_pool(name="psum_s", bufs=2, space="PSUM"))
        psum_b = ctx.enter_context(tc.tile_pool(name="psum_b", bufs=2, space="PSUM"))
        psum_m = ctx.enter_context(tc.tile_pool(name="psum_m", bufs=4, space="PSUM"))

        ones_col = const.tile([P, 1], dt.bfloat16)
        nc.vector.memset(ones_col, 1.0)
        ones_row = const.tile([1, P], dt.bfloat16)
        nc.vector.memset(ones_row, 1.0)

        # ---------------- mine prep: z_mine (lhsT layout) ----------------
        xm_s = prol.tile([P, CTILES, RPC], dt.bfloat16)
        nc.gpsimd.dma_start(out=xm_s, in_=xm_d[:])
        sqm = prol.tile([P, CTILES, RPC], dt.bfloat16)
        nc.vector.tensor_mul(sqm, xm_s, xm_s)
        zm = persist.tile([P, CTILES, RPC], dt.bfloat16)
        for h in range(RPC // NT):  # 2 halves of 512 cols
            hs = slice(h * NT, (h + 1) * NT)
            ps = psum_s.tile([1, NT], dt.float32)
            for c in range(CTILES):
                nc.tensor.matmul(ps, ones_col, sqm[:, c, hs],
                                 start=(c == 0), stop=(c == CTILES - 1))
            ln_m = small.tile([1, NT], dt.bfloat16)
            nc.scalar.activation(ln_m, ps, AF.Ln)
            pb = psum_b.tile([P, NT], dt.float32)
            nc.tensor.matmul(pb, ones_row, ln_m)
            rb_m = small.tile([P, NT], dt.bfloat16)
            nc.scalar.activation(rb_m, pb, AF.Exp, scale=-0.5)
            rb_m_b = bass.AP(tensor=rb_m.tensor, offset=rb_m.offset,
                             ap=[rb_m.ap[0], [0, CTILES], rb_m.ap[1]])
            nc.vector.tensor_mul(zm[:, :, hs], xm_s[:, :, hs], rb_m_b)

        # ---------------- T-side inputs (compute deferred past main loop) ----
        xmT_s = prol.tile([P, ITILES, D], dt.bfloat16)
        nc.gpsimd.dma_start(out=xmT_s, in_=xmT_d[:])
        xpT_s = prol.tile([P, ITILES, D], dt.bfloat16)
        nc.gpsimd.dma_start(out=xpT_s, in_=xpT_d[:])

        # ---------------- main loop over column tiles ----------------
        # xa staged in one persistent tile: 16 DMAs into disjoint n-slices,
        # fresh memory each -> at most 1 sem wait per DMA (HW limit).
        # Norm chain for tile n+1 is emitted BEFORE tile n's main matmuls so
        # the PE's in-order stream never stalls on the ACT/DVE chain.
        xa_f = persist.tile([P, NTILES, CTILES, NT], dt.bfloat16)
        rowpart = persist.tile([P, ITILES, NTILES], dt.float32)

        def norm_tile(n):
            nc.gpsimd.dma_start(out=xa_f[:, n, :, :], in_=xa_d[n])
            sq_n = nstream.tile([P, CTILES, NT], dt.bfloat16, name=f"sq_{n}",
                                tag="sq_n", bufs=3)
            nc.vector.tensor_mul(sq_n, xa_f[:, n, :, :], xa_f[:, n, :, :])
            ps = psum_s.tile([1, NT], dt.float32, name=f"ps_{n}", tag="ps")
            for c in range(CTILES):
                nc.tensor.matmul(ps, ones_col, sq_n[:, c, :],
                                 start=(c == 0), stop=(c == CTILES - 1))
            ln_n = small.tile([1, NT], dt.bfloat16, name=f"ln_{n}", tag="ln_n")
            nc.scalar.activation(ln_n, ps, AF.Ln)
            pb = psum_b.tile([P, NT], dt.float32, name=f"pb_{n}", tag="pb")
            nc.tensor.matmul(pb, ones_row, ln_n)
            rb_n = small.tile([P, NT], dt.bfloat16, name=f"rb_{n}", tag="rb_n")
            nc.scalar.activation(rb_n, pb, AF.Exp, scale=-0.5)
            rb_n_b = bass.AP(tensor=rb_n.tensor, offset=rb_n.offset,
                             ap=[rb_n.ap[0], [0, CTILES], rb_n.ap[1]])
            za_n = nstream.tile([P, CTILES, NT], dt.bfloat16, name=f"za_{n}",
                                tag="za_n", bufs=3)
            nc.vector.tensor_mul(za_n, xa_f[:, n, :, :], rb_n_b)
            return za_n

        za_tiles = {0: norm_tile(0), 1: norm_tile(1)}
        for n in range(NTILES):
            if n + 2 < NTILES:
                za_tiles[n + 2] = norm_tile(n + 2)
            za_n = za_tiles.pop(n)
            for i in range(ITILES):
                pm = psum_m.tile([P, NT], dt.float32, name=f"pm_{n}_{i}", tag="pm")
                for c in range(CTILES):
                    nc.tensor.matmul(pm, zm[:, c, i * P:(i + 1) * P], za_n[:, c, :],
                                     start=(c == 0), stop=(c == CTILES - 1))
                ej = junkp.tile([P, NT], dt.bfloat16, name=f"ej_{n}_{i}", tag="ej")
                nc.scalar.activation(ej, pm, AF.Exp, scale=INV_T,
                                     accum_out=rowpart[:, i, n:n + 1])

        # ---------------- T-side compute: positives & self terms -------------
        tmp8 = prol.tile([P, ITILES, D], dt.bfloat16)
        ssq_m = small.tile([P, ITILES], dt.float32)
        nc.vector.tensor_mul(tmp8, xmT_s, xmT_s)
        nc.vector.tensor_reduce(ssq_m, tmp8, axis=mybir.AxisListType.X, op=ALU.add)
        ssq_p = small.tile([P, ITILES], dt.float32)
        nc.vector.tensor_mul(tmp8, xpT_s, xpT_s)
        nc.vector.tensor_reduce(ssq_p, tmp8, axis=mybir.AxisListType.X, op=ALU.add)
        r_m = small.tile([P, ITILES], dt.float32)
        r_p = small.tile([P, ITILES], dt.float32)
        nc.scalar.activation(r_m, ssq_m, AF.Ln)
        nc.scalar.activation(r_m, r_m, AF.Exp, scale=-0.5)
        nc.scalar.activation(r_p, ssq_p, AF.Ln)
        nc.scalar.activation(r_p, r_p, AF.Exp, scale=-0.5)
        zmT = prol.tile([P, ITILES, D], dt.bfloat16)
        zpT = prol.tile([P, ITILES, D], dt.bfloat16)
        for i in range(ITILES):
            nc.vector.tensor_scalar_mul(zmT[:, i, :], xmT_s[:, i, :], r_m[:, i:i + 1])
            nc.vector.tensor_scalar_mul(zpT[:, i, :], xpT_s[:, i, :], r_p[:, i:i + 1])
        posT = small.tile([P, ITILES], dt.float32)
        nc.vector.tensor_mul(tmp8, zmT, zpT)
        nc.vector.tensor_reduce(posT, tmp8, axis=mybir.AxisListType.X, op=ALU.add)
        s2T = small.tile([P, ITILES], dt.float32)
        nc.vector.tensor_mul(tmp8, zmT, zmT)
        nc.vector.tensor_reduce(s2T, tmp8, axis=mybir.AxisListType.X, op=ALU.add)

        # ---------------- epilogue: per-row losses ----------------
        rowsum = small.tile([P, ITILES], dt.float32)
        nc.vector.tensor_reduce(rowsum, rowpart, axis=mybir.AxisListType.X,
                                op=ALU.add)
        selfe = small.tile([P, ITILES], dt.float32)
        nc.scalar.activation(selfe, s2T, AF.Exp, scale=INV_T)
        denom = small.tile([P, ITILES], dt.float32)
        nc.vector.tensor_sub(denom, rowsum, selfe)
        ld = small.tile([P, ITILES], dt.float32)
        nc.scalar.activation(ld, denom, AF.Ln)
        negpos = small.tile([P, ITILES], dt.float32)
        nc.vector.tensor_scalar_mul(negpos, posT, -INV_T)
        loss_t = small.tile([P, ITILES], dt.float32)
        nc.vector.tensor_add(loss_t, ld, negpos)
        nc.gpsimd.dma_start(out=out_d[:], in_=loss_t)

        dbg = small.tile([P, ITILES, 4], dt.float32)
        nc.vector.tensor_copy(dbg[:, :, 0], rowsum)
        nc.vector.tensor_copy(dbg[:, :, 1], denom)
        nc.vector.tensor_copy(dbg[:, :, 2], posT)
        nc.vector.tensor_copy(dbg[:, :, 3], s2T)
        nc.gpsimd.dma_start(out=dbg_d[:], in_=dbg)

    nc.compile()
    return nc


def _get_nc():
    if "nc" not in _CACHE:
        _CACHE["nc"] = _build_bass()
    return _CACHE["nc"]


def _prep_inputs(polyline_embs, c_embs):
    """Host-side shard/tile prep. Returns in_maps for the 8 cores."""
    bf16 = ml_dtypes.bfloat16
    z = np.concatenate([np.asarray(polyline_embs, np.float32),
                        np.asarray(c_embs, np.float32)], axis=0)  # [8192, 512]
    zb = z.astype(bf16)                                            # quantize once

    # xa: [512, 8192]^T tiled -> [n][p][c][col]
    xt = np.ascontiguousarray(zb.T)                                # [512, 8192]
    xa = np.ascontiguousarray(
        xt.reshape(CTILES, P, NTILES, NT).transpose(2, 1, 0, 3))   # [16,128,4,512]

    in_maps = []
    for k in range(NCORES):
        rows = zb[k * RPC:(k + 1) * RPC]                           # [1024, 512]
        prows_start = (k * RPC + B) % N
        prows = zb[prows_start:prows_start + RPC]
        xm = np.ascontiguousarray(
            rows.T.reshape(CTILES, P, RPC).transpose(1, 0, 2))     # [128, 4, 1024]
        xmT = np.ascontiguousarray(
            rows.reshape(ITILES, P, D).transpose(1, 0, 2))         # [128, 8, 512]
        xpT = np.ascontiguousarray(
            prows.reshape(ITILES, P, D).transpose(1, 0, 2))        # [128, 8, 512]
        in_maps.append({"xa": xa, "xm": xm, "xmT": xmT, "xpT": xpT})
    return in_maps


def kernel(polyline_embs, c_embs):
    from concourse.bass_utils import run_bass_kernel_spmd

    nc = _get_nc()
    in_maps = _prep_inputs(polyline_embs, c_embs)
    res = run_bass_kernel_spmd(nc, in_maps, core_ids=list(range(NCORES)))
    _CACHE["last_results"] = res
    total = 0.0
    for r in res.results:
        total += r["loss_rows"].astype(np.float64).sum()
    return np.float32(total / N)



# revision 13
# speedup vs baseline: 1.1499x; 1.1499x over previous
"""InfoNCE loss kernel for Trainium2, 8 NeuronCores (v3).

Reference computation:
    z = l2_normalize(concat([polyline_embs, c_embs]))   # [8192, 512]
    sim = z @ z.T                                       # [8192, 8192]
    denom_i = sum_{j != i} exp(sim_ij / T)
    pos_i   = sim[i, i +- B]
    loss    = mean(log(denom_i) - pos_i / T)

Design (per core, identical SPMD program):
  - Host ships bf16 x in a column-tiled layout [n][p][c][col], ROTATED
    per core so the core's own 1024 rows are local column tiles 0-1
    (row sums are column-order invariant).
  - Norms WITHOUT the scalar engine: sq = x*x (DVE, all-bf16 fast
    path), ssq via ones-matmuls, then rsqrt by a Quake-style Newton
    iteration on DVE, done on a [128, W]-shaped copy of ssq obtained
    via a DRAM round-trip (the [1, N] shape would serialize on one
    DVE lane). rb = 128/||x||.
  - za8 = e4m3(x * rb) = e4m3(128 z): DVE multiply against the
    matmul-broadcast rb, fp8 output.
  - Main: 6 column groups (5x3 + 1x1 tiles) x 8 row tiles: fp8
    DoubleRow matmuls (K=256) into a [128, 3*512] PSUM group, one exp
    activation per group with accum_out row sums.
    exp(psum * 2^-13) = exp(sim/T).
  - The scalar engine runs ONLY the main exps and one final Ln: two
    activation-table loads total.
  - Epilogue: denom = rowsum - e^2 (self-term is constant 1 up to
    quantization, error ~1e-5 of the ~8200 denominator); one Ln.
    Positives need no log: sum_i pos_i is a single DVE multiply +
    reduce over za8 (mine tiles 0-1 x partner tiles 8-9).
  - Host: loss = (sum ln denom - (sum pospart)*2^-14/T) / 8192.
"""

import numpy as np
import ml_dtypes

B = 4096
D = 512
N = 2 * B            # 8192 rows of sim
NCORES = 8
RPC = N // NCORES    # 1024 rows per core
P = 128              # partitions
NT = 512             # column-tile width
NTILES = N // NT     # 16 column tiles
CT = D // P          # 4 contraction chunks of 128
ITILES = RPC // P    # 8 row tiles per core
GROUPS = [3, 3, 3, 3, 3, 1]          # n-tiles per psum group (sum 16)
NGRP = len(GROUPS)
# ssq scaled by 2^-14 before rsqrt -> rb = 128/||x||, za8 = e4m3(128 z)
SSQ_SCALE = 2.0 ** -14
# psum = za8 . za8 = 2^14 z.z ; exp(psum * EXP_SCALE) = exp(sim / T), T=0.5
EXP_SCALE = 2.0 ** -13
POS_SCALE = 2.0 ** -14
SELF_TERM = float(np.exp(2.0))   # exp(sim_ii / T), sim_ii = 1
INV_T = 2.0
MAGIC1 = 0x5F3759E0              # rsqrt magic + 1 (for ~x + magic + 1)

_CACHE = {}


def _build_bass():
    """Trace the per-core Bass program (identical for all 8 cores)."""
    import concourse.bass as bass
    import concourse.tile as tile
    from concourse import bacc, mybir

    dt = mybir.dt
    AF = mybir.ActivationFunctionType
    ALU = mybir.AluOpType
    DR = mybir.MatmulPerfMode.DoubleRow

    nc = bacc.Bacc(None, target_bir_lowering=False, debug=False, num_swdge_queues=4)

    xa_d = nc.dram_tensor("xa", [NTILES, P, CT, NT], dt.bfloat16,
                          kind="ExternalInput")
    out_d = nc.dram_tensor("loss_rows", [P, ITILES], dt.float32,
                           kind="ExternalOutput")
    pp_d = nc.dram_tensor("pospart", [P, 1], dt.float32, kind="ExternalOutput")
    dbg_d = nc.dram_tensor("dbg", [P, ITILES, 2], dt.float32,
                           kind="ExternalOutput")
    # DRAM bounce buffers for the [1, N] <-> [128, N/128] reshape
    batches = [(0, 4), (4, NTILES)]
    scr_f = [nc.dram_tensor(f"scrf_{i}", [P, (b1 - b0) * NT // P], dt.float32,
                            kind="Internal") for i, (b0, b1) in enumerate(batches)]
    scr_b = [nc.dram_tensor(f"scrb_{i}", [P, (b1 - b0) * NT // P], dt.bfloat16,
                            kind="Internal") for i, (b0, b1) in enumerate(batches)]

    from contextlib import ExitStack

    with tile.TileContext(nc) as tc, ExitStack() as ctx:
        const = ctx.enter_context(tc.tile_pool(name="const", bufs=1))
        persist = ctx.enter_context(tc.tile_pool(name="persist", bufs=1))
        sqring = ctx.enter_context(tc.tile_pool(name="sqring", bufs=3))
        small = ctx.enter_context(tc.tile_pool(name="small", bufs=2))
        nwt = ctx.enter_context(tc.tile_pool(name="nwt", bufs=1))
        psum_pre = ctx.enter_context(tc.tile_pool(name="psum_pre", bufs=2,
                                                  space="PSUM"))
        psum_m = ctx.enter_context(tc.tile_pool(name="psum_m", bufs=2,
                                                space="PSUM"))

        ones_col = const.tile([P, 1], dt.bfloat16)
        nc.vector.memset(ones_col, 1.0)
        ones_row = const.tile([1, P], dt.bfloat16)
        nc.vector.memset(ones_row, 1.0)

        xa = persist.tile([P, NTILES, CT, NT], dt.bfloat16)
        za8 = persist.tile([P, NTILES, CT, NT], dt.float8e4)
        ssq_sb = persist.tile([1, N], dt.float32)
        rb_sb = persist.tile([1, N], dt.bfloat16)
        rowpart = persist.tile([P, ITILES, NGRP], dt.float32)

        # ---------------- input DMAs (all 16 tiles, pipelined) -----------
        for n in range(NTILES):
            nc.gpsimd.dma_start(out=xa[:, n, :, :], in_=xa_d[n])

        # ---------------- prologue helpers ------------------------------
        def ssq_tiles(tiles):
            """sq + ssq matmuls + scaled copy to ssq_sb for given tiles."""
            for n in tiles:
                sq = sqring.tile([P, CT, NT], dt.bfloat16,
                                 name=f"sq_{n}", tag="sq", bufs=3)
                nc.vector.tensor_mul(sq, xa[:, n, :, :], xa[:, n, :, :])
                ps = psum_pre.tile([P, NT], dt.float32,
                                   name=f"ssq_{n}", tag="pre")
                for c in range(CT):
                    nc.tensor.matmul(ps[0:1, :], ones_col, sq[:, c, :],
                                     start=(c == 0), stop=(c == CT - 1))
                nc.vector.tensor_scalar_mul(
                    ssq_sb[0:1, n * NT:(n + 1) * NT], ps[0:1, :], SSQ_SCALE)

        def newton(bi):
            """rb = rsqrt(ssq*2^-14) on a [128, W] reshape via DRAM."""
            b0, b1 = batches[bi]
            lo, hi = b0 * NT, b1 * NT
            W = (hi - lo) // P
            nc.gpsimd.dma_start(out=scr_f[bi][:], in_=ssq_sb[0:1, lo:hi])
            yt = nwt.tile([P, W], dt.float32, name=f"yt_{bi}")
            nc.gpsimd.dma_start(out=yt, in_=scr_f[bi][:])
            # y0 bits = MAGIC - (bits(y) >> 1), via float-domain value math
            # (DVE add is fp32 internally; the mod-2^32 trick would overflow)
            tu = nwt.tile([P, W], dt.uint32, name=f"tu_{bi}")
            nc.vector.tensor_scalar(tu, yt.bitcast(dt.uint32), 1, None,
                                    op0=ALU.arith_shift_right)
            tf = nwt.tile([P, W], dt.float32, name=f"tf_{bi}")
            nc.vector.tensor_copy(tf, tu)
            nc.vector.tensor_scalar(tf, tf, -1.0, float(MAGIC1 - 1),
                                    op0=ALU.mult, op1=ALU.add)
            y0u = nwt.tile([P, W], dt.uint32, name=f"y0u_{bi}")
            nc.vector.tensor_copy(y0u, tf)
            h = nwt.tile([P, W], dt.float32, name=f"h_{bi}")
            nc.vector.tensor_scalar_mul(h, yt, 0.5)
            y = y0u.bitcast(dt.float32)
            for it in range(2):
                t2 = nwt.tile([P, W], dt.float32, name=f"t2_{bi}_{it}")
                nc.vector.tensor_mul(t2, y, y)
                nc.vector.tensor_mul(t2, t2, h)
                nc.vector.tensor_scalar(t2, t2, -1.0, 1.5,
                                        op0=ALU.mult, op1=ALU.add)
                yn = nwt.tile([P, W], dt.float32, name=f"yn_{bi}_{it}")
                nc.vector.tensor_mul(yn, y, t2)
                y = yn
            rbT = nwt.tile([P, W], dt.bfloat16, name=f"rbT_{bi}")
            nc.vector.tensor_copy(rbT, y)
            nc.gpsimd.dma_start(out=scr_b[bi][:], in_=rbT)
            nc.gpsimd.dma_start(out=rb_sb[0:1, lo:hi], in_=scr_b[bi][:])

        def za8_tiles(tiles):
            """broadcast rb + quantize za8 for given tiles."""
            for n in tiles:
                rbb = psum_pre.tile([P, NT], dt.float32,
                                    name=f"rbb_{n}", tag="pre")
                nc.tensor.matmul(rbb, ones_row,
                                 rb_sb[0:1, n * NT:(n + 1) * NT])
                rb_bc = bass.AP(tensor=rbb.tensor, offset=rbb.offset,
                                ap=[rbb.ap[0], [0, CT], rbb.ap[1]])
                nc.vector.tensor_mul(za8[:, n, :, :], xa[:, n, :, :], rb_bc)

        def main_group(g, n0, gsz):
            for i in range(ITILES):
                pm = psum_m.tile([P, 3, NT], dt.float32,
                                 name=f"pm_{g}_{i}", tag="pm")
                nm, mc = divmod(i, 4)
                for gg in range(2):
                    lhsT = za8[:, nm, 2 * gg:2 * gg + 2, mc * P:(mc + 1) * P]
                    for nn in range(gsz):
                        nc.tensor.matmul(pm[:, nn, :], lhsT,
                                         za8[:, n0 + nn, 2 * gg:2 * gg + 2, :],
                                         start=(gg == 0), stop=(gg == 1),
                                         perf_mode=DR)
                ej = sqring.tile([P, 3, NT], dt.bfloat16,
                                 name=f"ej_{g}_{i}", tag="ej", bufs=2)
                nc.scalar.activation(ej[:, 0:gsz, :], pm[:, 0:gsz, :], AF.Exp,
                                     scale=EXP_SCALE,
                                     accum_out=rowpart[:, i, g:g + 1])

        # ---------------- phase order ------------------------------------
        ssq_tiles(range(0, 4))          # batch A norms
        newton(0)
        za8_tiles(range(0, 4))

        gstart = np.cumsum([0] + GROUPS).tolist()
        main_group(0, gstart[0], GROUPS[0])   # tiles 0-2 while B norms run

        ssq_tiles(range(4, NTILES))     # batch B norms
        newton(1)
        za8_tiles(range(4, NTILES))

        for g in range(1, NGRP):
            main_group(g, gstart[g], GROUPS[g])

        # ---------------- positives partial (no log needed) --------------
        junk = small.tile([P, 2, CT, NT], dt.bfloat16)
        nc.vector.tensor_mul(junk, za8[:, 0:2, :, :], za8[:, 8:10, :, :])
        jf = bass.AP(tensor=junk.tensor, offset=junk.offset,
                     ap=[junk.ap[0], [1, 2 * CT * NT]])
        pp = small.tile([P, 1], dt.float32)
        nc.vector.tensor_reduce(pp, jf, axis=mybir.AxisListType.X, op=ALU.add)
        nc.gpsimd.dma_start(out=pp_d[:], in_=pp)

        # ---------------- epilogue: per-row log denominators -------------
        rowsum = small.tile([P, ITILES], dt.float32)
        nc.vector.tensor_reduce(rowsum, rowpart, axis=mybir.AxisListType.X,
                                op=ALU.add)
        denom = small.tile([P, ITILES], dt.float32)
        nc.vector.tensor_scalar_add(denom, rowsum, -SELF_TERM)
        loss_t = small.tile([P, ITILES], dt.float32)
        nc.scalar.activation(loss_t, denom, AF.Ln)
        nc.gpsimd.dma_start(out=out_d[:], in_=loss_t)

        dbg = small.tile([P, ITILES, 2], dt.float32)
        nc.vector.tensor_copy(dbg[:, :, 0], rowsum)
        nc.vector.tensor_copy(dbg[:, :, 1], denom)
        nc.gpsimd.dma_start(out=dbg_d[:], in_=dbg)

    nc.compile()
    return nc


def _get_nc():
    if "nc" not in _CACHE:
        _CACHE["nc"] = _build_bass()
    return _CACHE["nc"]


def _prep_inputs(polyline_embs, c_embs):
    """Host-side shard/tile prep. Returns in_maps for the 8 cores."""
    bf16 = ml_dtypes.bfloat16
    x = np.concatenate([np.asarray(polyline_embs, np.float32),
                        np.asarray(c_embs, np.float32)], axis=0)  # [8192, 512]
    xb = x.astype(bf16)

    in_maps = []
    for k in range(NCORES):
        qk = np.roll(xb, -k * RPC, axis=0)                        # mine first
        xt = np.ascontiguousarray(qk.T)                           # [512, 8192]
        xa = np.ascontiguousarray(
            xt.reshape(CT, P, NTILES, NT).transpose(2, 1, 0, 3))  # [16,128,4,512]
        in_maps.append({"xa": xa})
    return in_maps


def kernel(polyline_embs, c_embs):
    from concourse.bass_utils import run_bass_kernel_spmd

    nc = _get_nc()
    in_maps = _prep_inputs(polyline_embs, c_embs)
    res = run_bass_kernel_spmd(nc, in_maps, core_ids=list(range(NCORES)))
    _CACHE["last_results"] = res
    ln_total = 0.0
    pos_total = 0.0
    for r in res.results:
        ln_total += r["loss_rows"].astype(np.float64).sum()
        pos_total += r["pospart"].astype(np.float64).sum()
    loss = (ln_total - pos_total * POS_SCALE * INV_T) / N
    return np.float32(loss)


# revision 16
# speedup vs baseline: 1.2437x; 1.0816x over previous
"""InfoNCE loss kernel for Trainium2, 8 NeuronCores (v3).

Reference computation:
    z = l2_normalize(concat([polyline_embs, c_embs]))   # [8192, 512]
    sim = z @ z.T                                       # [8192, 8192]
    denom_i = sum_{j != i} exp(sim_ij / T)
    pos_i   = sim[i, i +- B]
    loss    = mean(log(denom_i) - pos_i / T)

Design (per core, identical SPMD program):
  - Host ships bf16 x in a column-tiled layout [n][p][c][col], ROTATED
    per core so the core's own 1024 rows are local column tiles 0-1
    (row sums are column-order invariant).
  - Norms WITHOUT the scalar engine: sq = x*x (DVE, all-bf16 fast
    path), ssq via ones-matmuls, then rsqrt by a Quake-style Newton
    iteration on DVE, done on a [128, W]-shaped copy of ssq obtained
    via a DRAM round-trip (the [1, N] shape would serialize on one
    DVE lane). rb = 128/||x||.
  - za8 = e4m3(x * rb) = e4m3(128 z): DVE multiply against the
    matmul-broadcast rb, fp8 output.
  - Main: 6 column groups (5x3 + 1x1 tiles) x 8 row tiles: fp8
    DoubleRow matmuls (K=256) into a [128, 3*512] PSUM group, one exp
    activation per group with accum_out row sums.
    exp(psum * 2^-13) = exp(sim/T).
  - The scalar engine runs ONLY the main exps and one final Ln: two
    activation-table loads total.
  - Epilogue: denom = rowsum - e^2 (self-term is constant 1 up to
    quantization, error ~1e-5 of the ~8200 denominator); one Ln.
    Positives need no log: sum_i pos_i is a single DVE multiply +
    reduce over za8 (mine tiles 0-1 x partner tiles 8-9).
  - Host: loss = (sum ln denom - (sum pospart)*2^-14/T) / 8192.
"""

import numpy as np
import ml_dtypes

B = 4096
D = 512
N = 2 * B            # 8192 rows of sim
NCORES = 8
RPC = N // NCORES    # 1024 rows per core
P = 128              # partitions
NT = 512             # column-tile width
NTILES = N // NT     # 16 column tiles
CT = D // P          # 4 contraction chunks of 128
ITILES = RPC // P    # 8 row tiles per core
GROUPS = [3, 3, 3, 3, 3, 1]          # n-tiles per psum group (sum 16)
NGRP = len(GROUPS)
# ssq scaled by 2^-14 before rsqrt -> rb = 128/||x||, za8 = e4m3(128 z)
SSQ_SCALE = 2.0 ** -14
# psum = za8 . za8 = 2^14 z.z ; exp(psum * EXP_SCALE) = exp(sim / T), T=0.5
EXP_SCALE = 2.0 ** -13
POS_SCALE = 2.0 ** -14
SELF_TERM = float(np.exp(2.0))   # exp(sim_ii / T), sim_ii = 1
INV_T = 2.0
MAGIC1 = 0x5F3759E0              # rsqrt magic + 1 (for ~x + magic + 1)

_CACHE = {}


def _build_bass():
    """Trace the per-core Bass program (identical for all 8 cores)."""
    import concourse.bass as bass
    import concourse.tile as tile
    from concourse import bacc, mybir

    dt = mybir.dt
    AF = mybir.ActivationFunctionType
    ALU = mybir.AluOpType
    DR = mybir.MatmulPerfMode.DoubleRow

    nc = bacc.Bacc(None, target_bir_lowering=False, debug=False, num_swdge_queues=4)

    xa_d = nc.dram_tensor("xa", [NTILES, P, CT, NT], dt.bfloat16,
                          kind="ExternalInput")
    out_d = nc.dram_tensor("loss_rows", [P, ITILES], dt.float32,
                           kind="ExternalOutput")
    pp_d = nc.dram_tensor("pospart", [P, 1], dt.float32, kind="ExternalOutput")
    dbg_d = nc.dram_tensor("dbg", [P, ITILES, 2], dt.float32,
                           kind="ExternalOutput")
    # DRAM bounce buffers for the [1, 2048] <-> [16, 128] reshape
    # (16 rows keeps DMA descriptors big; 128 partitions would mean 128
    # tiny descriptors and ~10us per bounce)
    batches = [(0, 4), (4, 8), (8, 12), (12, NTILES)]
    BP = 16                                   # bounce partitions
    scr_f = [nc.dram_tensor(f"scrf_{i}", [BP, (b1 - b0) * NT // BP], dt.float32,
                            kind="Internal") for i, (b0, b1) in enumerate(batches)]
    scr_b = [nc.dram_tensor(f"scrb_{i}", [BP, (b1 - b0) * NT // BP], dt.bfloat16,
                            kind="Internal") for i, (b0, b1) in enumerate(batches)]

    from contextlib import ExitStack

    with tile.TileContext(nc) as tc, ExitStack() as ctx:
        const = ctx.enter_context(tc.tile_pool(name="const", bufs=1))
        persist = ctx.enter_context(tc.tile_pool(name="persist", bufs=1))
        sqring = ctx.enter_context(tc.tile_pool(name="sqring", bufs=3))
        small = ctx.enter_context(tc.tile_pool(name="small", bufs=2))
        nwt = ctx.enter_context(tc.tile_pool(name="nwt", bufs=1))
        psum_pre = ctx.enter_context(tc.tile_pool(name="psum_pre", bufs=2,
                                                  space="PSUM"))
        psum_m = ctx.enter_context(tc.tile_pool(name="psum_m", bufs=2,
                                                space="PSUM"))

        ones_col = const.tile([P, 1], dt.bfloat16)
        nc.vector.memset(ones_col, 1.0)
        ones_row = const.tile([1, P], dt.bfloat16)
        nc.vector.memset(ones_row, 1.0)

        xa = persist.tile([P, NTILES, CT, NT], dt.bfloat16)
        za8 = persist.tile([P, NTILES, CT, NT], dt.float8e4)
        ssq_sb = persist.tile([1, N], dt.float32)
        rb_sb = persist.tile([1, N], dt.bfloat16)
        rowpart = persist.tile([P, ITILES, NGRP], dt.float32)

        # ---------------- input DMAs (all 16 tiles, pipelined) -----------
        for n in range(NTILES):
            nc.gpsimd.dma_start(out=xa[:, n, :, :], in_=xa_d[n])

        # ---------------- prologue helpers ------------------------------
        def ssq_tiles(tiles):
            """sq + ssq matmuls + scaled copy to ssq_sb for given tiles."""
            for n in tiles:
                sq = sqring.tile([P, CT, NT], dt.bfloat16,
                                 name=f"sq_{n}", tag="sq", bufs=3)
                nc.vector.tensor_mul(sq, xa[:, n, :, :], xa[:, n, :, :])
                ps = psum_pre.tile([P, NT], dt.float32,
                                   name=f"ssq_{n}", tag="pre")
                for c in range(CT):
                    nc.tensor.matmul(ps[0:1, :], ones_col, sq[:, c, :],
                                     start=(c == 0), stop=(c == CT - 1))
                nc.vector.tensor_scalar_mul(
                    ssq_sb[0:1, n * NT:(n + 1) * NT], ps[0:1, :], SSQ_SCALE)

        def newton(bi):
            """rb = rsqrt(ssq*2^-14) on a [16, W] reshape via DRAM."""
            b0, b1 = batches[bi]
            lo, hi = b0 * NT, b1 * NT
            W = (hi - lo) // BP
            nc.gpsimd.dma_start(out=scr_f[bi][:], in_=ssq_sb[0:1, lo:hi])
            yt = nwt.tile([BP, W], dt.float32, name=f"yt_{bi}")
            nc.gpsimd.dma_start(out=yt, in_=scr_f[bi][:])
            # y0 bits = MAGIC - (bits(y) >> 1), via float-domain value math
            # (DVE add is fp32 internally; the mod-2^32 trick would overflow)
            tu = nwt.tile([BP, W], dt.uint32, name=f"tu_{bi}")
            nc.vector.tensor_scalar(tu, yt.bitcast(dt.uint32), 1, None,
                                    op0=ALU.arith_shift_right)
            tf = nwt.tile([BP, W], dt.float32, name=f"tf_{bi}")
            nc.vector.tensor_copy(tf, tu)
            nc.vector.tensor_scalar(tf, tf, -1.0, float(MAGIC1 - 1),
                                    op0=ALU.mult, op1=ALU.add)
            y0u = nwt.tile([BP, W], dt.uint32, name=f"y0u_{bi}")
            nc.vector.tensor_copy(y0u, tf)
            h = nwt.tile([BP, W], dt.float32, name=f"h_{bi}")
            nc.vector.tensor_scalar_mul(h, yt, 0.5)
            y = y0u.bitcast(dt.float32)
            for it in range(2):
                t2 = nwt.tile([BP, W], dt.float32, name=f"t2_{bi}_{it}")
                nc.vector.tensor_mul(t2, y, y)
                nc.vector.tensor_mul(t2, t2, h)
                nc.vector.tensor_scalar(t2, t2, -1.0, 1.5,
                                        op0=ALU.mult, op1=ALU.add)
                yn = nwt.tile([BP, W], dt.float32, name=f"yn_{bi}_{it}")
                nc.vector.tensor_mul(yn, y, t2)
                y = yn
            rbT = nwt.tile([BP, W], dt.bfloat16, name=f"rbT_{bi}")
            nc.vector.tensor_copy(rbT, y)
            nc.gpsimd.dma_start(out=scr_b[bi][:], in_=rbT)
            nc.gpsimd.dma_start(out=rb_sb[0:1, lo:hi], in_=scr_b[bi][:])

        def za8_tiles(tiles):
            """broadcast rb + quantize za8 for given tiles."""
            for n in tiles:
                rbb = psum_pre.tile([P, NT], dt.float32,
                                    name=f"rbb_{n}", tag="pre")
                nc.tensor.matmul(rbb, ones_row,
                                 rb_sb[0:1, n * NT:(n + 1) * NT])
                rb_bc = bass.AP(tensor=rbb.tensor, offset=rbb.offset,
                                ap=[rbb.ap[0], [0, CT], rbb.ap[1]])
                nc.vector.tensor_mul(za8[:, n, :, :], xa[:, n, :, :], rb_bc)

        def main_group(g, n0, gsz):
            for i in range(ITILES):
                pm = psum_m.tile([P, 3, NT], dt.float32,
                                 name=f"pm_{g}_{i}", tag="pm")
                nm, mc = divmod(i, 4)
                for gg in range(2):
                    lhsT = za8[:, nm, 2 * gg:2 * gg + 2, mc * P:(mc + 1) * P]
                    for nn in range(gsz):
                        nc.tensor.matmul(pm[:, nn, :], lhsT,
                                         za8[:, n0 + nn, 2 * gg:2 * gg + 2, :],
                                         start=(gg == 0), stop=(gg == 1),
                                         perf_mode=DR)
                ej = sqring.tile([P, 3, NT], dt.bfloat16,
                                 name=f"ej_{g}_{i}", tag="ej", bufs=2)
                nc.scalar.activation(ej[:, 0:gsz, :], pm[:, 0:gsz, :], AF.Exp,
                                     scale=EXP_SCALE,
                                     accum_out=rowpart[:, i, g:g + 1])

        # ---------------- phase order ------------------------------------
        # Interleave 4 norm batches with the 6 main groups so the norm
        # chain of batch b+1 overlaps main compute on earlier tiles.
        gstart = np.cumsum([0] + GROUPS).tolist()

        ssq_tiles(range(*batches[0]))
        ssq_tiles(range(*batches[1]))
        newton(0)
        za8_tiles(range(*batches[0]))
        newton(1)
        main_group(0, gstart[0], GROUPS[0])     # tiles 0-2
        za8_tiles(range(*batches[1]))
        ssq_tiles(range(*batches[2]))
        newton(2)
        main_group(1, gstart[1], GROUPS[1])     # tiles 3-5
        za8_tiles(range(*batches[2]))
        ssq_tiles(range(*batches[3]))
        newton(3)
        main_group(2, gstart[2], GROUPS[2])     # tiles 6-8
        za8_tiles(range(*batches[3]))
        for g in range(3, NGRP):
            main_group(g, gstart[g], GROUPS[g])

        # ---------------- positives partial (no log needed) --------------
        junk = small.tile([P, 2, CT, NT], dt.bfloat16)
        nc.vector.tensor_mul(junk, za8[:, 0:2, :, :], za8[:, 8:10, :, :])
        jf = bass.AP(tensor=junk.tensor, offset=junk.offset,
                     ap=[junk.ap[0], [1, 2 * CT * NT]])
        pp = small.tile([P, 1], dt.float32)
        nc.vector.tensor_reduce(pp, jf, axis=mybir.AxisListType.X, op=ALU.add)
        nc.gpsimd.dma_start(out=pp_d[:], in_=pp)

        # ---------------- epilogue: per-row log denominators -------------
        rowsum = small.tile([P, ITILES], dt.float32)
        nc.vector.tensor_reduce(rowsum, rowpart, axis=mybir.AxisListType.X,
                                op=ALU.add)
        denom = small.tile([P, ITILES], dt.float32)
        nc.vector.tensor_scalar_add(denom, rowsum, -SELF_TERM)
        loss_t = small.tile([P, ITILES], dt.float32)
        nc.scalar.activation(loss_t, denom, AF.Ln)
        nc.gpsimd.dma_start(out=out_d[:], in_=loss_t)

        dbg = small.tile([P, ITILES, 2], dt.float32)
        nc.vector.tensor_copy(dbg[:, :, 0], rowsum)
        nc.vector.tensor_copy(dbg[:, :, 1], denom)
        nc.gpsimd.dma_start(out=dbg_d[:], in_=dbg)

    nc.compile()
    return nc


def _get_nc():
    if "nc" not in _CACHE:
        _CACHE["nc"] = _build_bass()
    return _CACHE["nc"]


def _prep_inputs(polyline_embs, c_embs):
    """Host-side shard/tile prep. Returns in_maps for the 8 cores."""
    bf16 = ml_dtypes.bfloat16
    x = np.concatenate([np.asarray(polyline_embs, np.float32),
                        np.asarray(c_embs, np.float32)], axis=0)  # [8192, 512]
    xb = x.astype(bf16)

    in_maps = []
    for k in range(NCORES):
        qk = np.roll(xb, -k * RPC, axis=0)                        # mine first
        xt = np.ascontiguousarray(qk.T)                           # [512, 8192]
        xa = np.ascontiguousarray(
            xt.reshape(CT, P, NTILES, NT).transpose(2, 1, 0, 3))  # [16,128,4,512]
        in_maps.append({"xa": xa})
    return in_maps


def kernel(polyline_embs, c_embs):
    from concourse.bass_utils import run_bass_kernel_spmd

    nc = _get_nc()
    in_maps = _prep_inputs(polyline_embs, c_embs)
    res = run_bass_kernel_spmd(nc, in_maps, core_ids=list(range(NCORES)))
    _CACHE["last_results"] = res
    ln_total = 0.0
    pos_total = 0.0
    for r in res.results:
        ln_total += r["loss_rows"].astype(np.float64).sum()
        pos_total += r["pospart"].astype(np.float64).sum()
    loss = (ln_total - pos_total * POS_SCALE * INV_T) / N
    return np.float32(loss)


# revision 18
# speedup vs baseline: 1.2687x; 1.0201x over previous
"""InfoNCE loss kernel for Trainium2, 8 NeuronCores (v3).

Reference computation:
    z = l2_normalize(concat([polyline_embs, c_embs]))   # [8192, 512]
    sim = z @ z.T                                       # [8192, 8192]
    denom_i = sum_{j != i} exp(sim_ij / T)
    pos_i   = sim[i, i +- B]
    loss    = mean(log(denom_i) - pos_i / T)

Design (per core, identical SPMD program):
  - Host ships bf16 x in a column-tiled layout [n][p][c][col], ROTATED
    per core so the core's own 1024 rows are local column tiles 0-1
    (row sums are column-order invariant).
  - Norms WITHOUT the scalar engine: sq = x*x (DVE, all-bf16 fast
    path), ssq via ones-matmuls, then rsqrt by a Quake-style Newton
    iteration on DVE, done on a [128, W]-shaped copy of ssq obtained
    via a DRAM round-trip (the [1, N] shape would serialize on one
    DVE lane). rb = 128/||x||.
  - za8 = e4m3(x * rb) = e4m3(128 z): DVE multiply against the
    matmul-broadcast rb, fp8 output.
  - Main: 6 column groups (5x3 + 1x1 tiles) x 8 row tiles: fp8
    DoubleRow matmuls (K=256) into a [128, 3*512] PSUM group, one exp
    activation per group with accum_out row sums.
    exp(psum * 2^-13) = exp(sim/T).
  - The scalar engine runs ONLY the main exps and one final Ln: two
    activation-table loads total.
  - Epilogue: denom = rowsum - e^2 (self-term is constant 1 up to
    quantization, error ~1e-5 of the ~8200 denominator); one Ln.
    Positives need no log: sum_i pos_i is a single DVE multiply +
    reduce over za8 (mine tiles 0-1 x partner tiles 8-9).
  - Host: loss = (sum ln denom - (sum pospart)*2^-14/T) / 8192.
"""

import numpy as np
import ml_dtypes

B = 4096
D = 512
N = 2 * B            # 8192 rows of sim
NCORES = 8
RPC = N // NCORES    # 1024 rows per core
P = 128              # partitions
NT = 512             # column-tile width
NTILES = N // NT     # 16 column tiles
CT = D // P          # 4 contraction chunks of 128
ITILES = RPC // P    # 8 row tiles per core
GROUPS = [3, 3, 3, 3, 3, 1]          # n-tiles per psum group (sum 16)
NGRP = len(GROUPS)
# ssq scaled by 2^-14 before rsqrt -> rb = 128/||x||, za8 = e4m3(128 z)
SSQ_SCALE = 2.0 ** -14
# psum = za8 . za8 = 2^14 z.z ; exp(psum * EXP_SCALE) = exp(sim / T), T=0.5
EXP_SCALE = 2.0 ** -13
POS_SCALE = 2.0 ** -14
SELF_TERM = float(np.exp(2.0))   # exp(sim_ii / T), sim_ii = 1
INV_T = 2.0
MAGIC1 = 0x5F3759E0              # rsqrt magic + 1 (for ~x + magic + 1)

_CACHE = {}


def _build_bass():
    """Trace the per-core Bass program (identical for all 8 cores)."""
    import concourse.bass as bass
    import concourse.tile as tile
    from concourse import bacc, mybir

    dt = mybir.dt
    AF = mybir.ActivationFunctionType
    ALU = mybir.AluOpType
    DR = mybir.MatmulPerfMode.DoubleRow

    nc = bacc.Bacc(None, target_bir_lowering=False, debug=False, num_swdge_queues=4)

    xa_d = nc.dram_tensor("xa", [NTILES, P, CT, NT], dt.bfloat16,
                          kind="ExternalInput")
    out_d = nc.dram_tensor("loss_rows", [P, ITILES], dt.float32,
                           kind="ExternalOutput")
    pp_d = nc.dram_tensor("pospart", [P, 1], dt.float32, kind="ExternalOutput")
    dbg_d = nc.dram_tensor("dbg", [P, ITILES, 2], dt.float32,
                           kind="ExternalOutput")
    # DRAM bounce buffers for the [1, 2048] <-> [16, 128] reshape
    # (16 rows keeps DMA descriptors big; 128 partitions would mean 128
    # tiny descriptors and ~10us per bounce)
    batches = [(0, 4), (4, 8), (8, 12), (12, NTILES)]
    BP = 16                                   # bounce partitions
    scr_f = [nc.dram_tensor(f"scrf_{i}", [BP, (b1 - b0) * NT // BP], dt.float32,
                            kind="Internal") for i, (b0, b1) in enumerate(batches)]
    scr_b = [nc.dram_tensor(f"scrb_{i}", [BP, (b1 - b0) * NT // BP], dt.bfloat16,
                            kind="Internal") for i, (b0, b1) in enumerate(batches)]

    from contextlib import ExitStack

    with tile.TileContext(nc) as tc, ExitStack() as ctx:
        const = ctx.enter_context(tc.tile_pool(name="const", bufs=1))
        persist = ctx.enter_context(tc.tile_pool(name="persist", bufs=1))
        sqring = ctx.enter_context(tc.tile_pool(name="sqring", bufs=3))
        small = ctx.enter_context(tc.tile_pool(name="small", bufs=2))
        nwt = ctx.enter_context(tc.tile_pool(name="nwt", bufs=1))
        psum_pre = ctx.enter_context(tc.tile_pool(name="psum_pre", bufs=2,
                                                  space="PSUM"))
        psum_m = ctx.enter_context(tc.tile_pool(name="psum_m", bufs=2,
                                                space="PSUM"))

        ones_col = const.tile([P, 1], dt.bfloat16)
        nc.vector.memset(ones_col, 1.0)
        ones_row = const.tile([1, P], dt.bfloat16)
        nc.vector.memset(ones_row, 1.0)

        xa = persist.tile([P, NTILES, CT, NT], dt.bfloat16)
        za8 = persist.tile([P, NTILES, CT, NT], dt.float8e4)
        ssq_sb = persist.tile([1, N], dt.float32)
        rb_sb = persist.tile([1, N], dt.bfloat16)
        rowpart = persist.tile([P, ITILES, NGRP], dt.float32)

        # ---------------- input DMAs (all 16 tiles, pipelined) -----------
        # HWDGE via the idle sync engine: cheap triggers, and keeps the
        # gpsimd SWDGE queues free for the low-latency bounce DMAs.
        for n in range(NTILES):
            nc.sync.dma_start(out=xa[:, n, :, :], in_=xa_d[n])

        # ---------------- prologue helpers ------------------------------
        def ssq_tiles(tiles):
            """sq + ssq matmuls + scaled copy to ssq_sb for given tiles."""
            for n in tiles:
                sq = sqring.tile([P, CT, NT], dt.bfloat16,
                                 name=f"sq_{n}", tag="sq", bufs=3)
                nc.vector.tensor_mul(sq, xa[:, n, :, :], xa[:, n, :, :])
                ps = psum_pre.tile([P, NT], dt.float32,
                                   name=f"ssq_{n}", tag="pre")
                for c in range(CT):
                    nc.tensor.matmul(ps[0:1, :], ones_col, sq[:, c, :],
                                     start=(c == 0), stop=(c == CT - 1))
                nc.vector.tensor_scalar_mul(
                    ssq_sb[0:1, n * NT:(n + 1) * NT], ps[0:1, :], SSQ_SCALE)

        def newton(bi):
            """rb = rsqrt(ssq*2^-14) on a [16, W] reshape via DRAM."""
            b0, b1 = batches[bi]
            lo, hi = b0 * NT, b1 * NT
            W = (hi - lo) // BP
            nc.gpsimd.dma_start(out=scr_f[bi][:], in_=ssq_sb[0:1, lo:hi])
            yt = nwt.tile([BP, W], dt.float32, name=f"yt_{bi}")
            nc.gpsimd.dma_start(out=yt, in_=scr_f[bi][:])
            # y0 bits = MAGIC - (bits(y) >> 1), via float-domain value math
            # (DVE add is fp32 internally; the mod-2^32 trick would overflow)
            tu = nwt.tile([BP, W], dt.uint32, name=f"tu_{bi}")
            nc.vector.tensor_scalar(tu, yt.bitcast(dt.uint32), 1, None,
                                    op0=ALU.arith_shift_right)
            tf = nwt.tile([BP, W], dt.float32, name=f"tf_{bi}")
            nc.vector.tensor_copy(tf, tu)
            nc.vector.tensor_scalar(tf, tf, -1.0, float(MAGIC1 - 1),
                                    op0=ALU.mult, op1=ALU.add)
            y0u = nwt.tile([BP, W], dt.uint32, name=f"y0u_{bi}")
            nc.vector.tensor_copy(y0u, tf)
            h = nwt.tile([BP, W], dt.float32, name=f"h_{bi}")
            nc.vector.tensor_scalar_mul(h, yt, 0.5)
            y = y0u.bitcast(dt.float32)
            for it in range(2):
                t2 = nwt.tile([BP, W], dt.float32, name=f"t2_{bi}_{it}")
                nc.vector.tensor_mul(t2, y, y)
                nc.vector.tensor_mul(t2, t2, h)
                nc.vector.tensor_scalar(t2, t2, -1.0, 1.5,
                                        op0=ALU.mult, op1=ALU.add)
                yn = nwt.tile([BP, W], dt.float32, name=f"yn_{bi}_{it}")
                nc.vector.tensor_mul(yn, y, t2)
                y = yn
            rbT = nwt.tile([BP, W], dt.bfloat16, name=f"rbT_{bi}")
            nc.vector.tensor_copy(rbT, y)
            nc.gpsimd.dma_start(out=scr_b[bi][:], in_=rbT)
            nc.gpsimd.dma_start(out=rb_sb[0:1, lo:hi], in_=scr_b[bi][:])

        def za8_tiles(tiles):
            """broadcast rb + quantize za8 for given tiles."""
            for n in tiles:
                rbb = psum_pre.tile([P, NT], dt.float32,
                                    name=f"rbb_{n}", tag="pre")
                nc.tensor.matmul(rbb, ones_row,
                                 rb_sb[0:1, n * NT:(n + 1) * NT])
                rb_bc = bass.AP(tensor=rbb.tensor, offset=rbb.offset,
                                ap=[rbb.ap[0], [0, CT], rbb.ap[1]])
                nc.vector.tensor_mul(za8[:, n, :, :], xa[:, n, :, :], rb_bc)

        def main_group(g, n0, gsz):
            for i in range(ITILES):
                pm = psum_m.tile([P, 3, NT], dt.float32,
                                 name=f"pm_{g}_{i}", tag="pm")
                nm, mc = divmod(i, 4)
                for gg in range(2):
                    lhsT = za8[:, nm, 2 * gg:2 * gg + 2, mc * P:(mc + 1) * P]
                    for nn in range(gsz):
                        nc.tensor.matmul(pm[:, nn, :], lhsT,
                                         za8[:, n0 + nn, 2 * gg:2 * gg + 2, :],
                                         start=(gg == 0), stop=(gg == 1),
                                         perf_mode=DR)
                ej = sqring.tile([P, 3, NT], dt.bfloat16,
                                 name=f"ej_{g}_{i}", tag="ej", bufs=2)
                nc.scalar.activation(ej[:, 0:gsz, :], pm[:, 0:gsz, :], AF.Exp,
                                     scale=EXP_SCALE,
                                     accum_out=rowpart[:, i, g:g + 1])

        # ---------------- phase order ------------------------------------
        # Interleave 4 norm batches with the 6 main groups so the norm
        # chain of batch b+1 overlaps main compute on earlier tiles.
        gstart = np.cumsum([0] + GROUPS).tolist()

        ssq_tiles(range(*batches[0]))
        newton(0)
        ssq_tiles(range(*batches[1]))
        newton(1)
        za8_tiles(range(*batches[0]))
        main_group(0, gstart[0], GROUPS[0])     # tiles 0-2
        za8_tiles(range(*batches[1]))
        main_group(1, gstart[1], GROUPS[1])     # tiles 3-5
        ssq_tiles(range(*batches[2]))
        newton(2)
        za8_tiles(range(*batches[2]))
        main_group(2, gstart[2], GROUPS[2])     # tiles 6-8
        ssq_tiles(range(*batches[3]))
        newton(3)
        za8_tiles(range(*batches[3]))
        for g in range(3, NGRP):
            main_group(g, gstart[g], GROUPS[g])

        # ---------------- positives partial (no log needed) --------------
        junk = small.tile([P, 2, CT, NT], dt.bfloat16)
        nc.vector.tensor_mul(junk, za8[:, 0:2, :, :], za8[:, 8:10, :, :])
        jf = bass.AP(tensor=junk.tensor, offset=junk.offset,
                     ap=[junk.ap[0], [1, 2 * CT * NT]])
        pp = small.tile([P, 1], dt.float32)
        nc.vector.tensor_reduce(pp, jf, axis=mybir.AxisListType.X, op=ALU.add)
        nc.gpsimd.dma_start(out=pp_d[:], in_=pp)

        # ---------------- epilogue: per-row log denominators -------------
        rowsum = small.tile([P, ITILES], dt.float32)
        nc.vector.tensor_reduce(rowsum, rowpart, axis=mybir.AxisListType.X,
                                op=ALU.add)
        denom = small.tile([P, ITILES], dt.float32)
        nc.vector.tensor_scalar_add(denom, rowsum, -SELF_TERM)
        loss_t = small.tile([P, ITILES], dt.float32)
        nc.scalar.activation(loss_t, denom, AF.Ln)
        nc.gpsimd.dma_start(out=out_d[:], in_=loss_t)

        dbg = small.tile([P, ITILES, 2], dt.float32)
        nc.vector.tensor_copy(dbg[:, :, 0], rowsum)
        nc.vector.tensor_copy(dbg[:, :, 1], denom)
        nc.gpsimd.dma_start(out=dbg_d[:], in_=dbg)

    nc.compile()
    return nc


def _get_nc():
    if "nc" not in _CACHE:
        _CACHE["nc"] = _build_bass()
    return _CACHE["nc"]


def _prep_inputs(polyline_embs, c_embs):
    """Host-side shard/tile prep. Returns in_maps for the 8 cores."""
    bf16 = ml_dtypes.bfloat16
    x = np.concatenate([np.asarray(polyline_embs, np.float32),
                        np.asarray(c_embs, np.float32)], axis=0)  # [8192, 512]
    xb = x.astype(bf16)

    in_maps = []
    for k in range(NCORES):
        qk = np.roll(xb, -k * RPC, axis=0)                        # mine first
        xt = np.ascontiguousarray(qk.T)                           # [512, 8192]
        xa = np.ascontiguousarray(
            xt.reshape(CT, P, NTILES, NT).transpose(2, 1, 0, 3))  # [16,128,4,512]
        in_maps.append({"xa": xa})
    return in_maps


def kernel(polyline_embs, c_embs):
    from concourse.bass_utils import run_bass_kernel_spmd

    nc = _get_nc()
    in_maps = _prep_inputs(polyline_embs, c_embs)
    res = run_bass_kernel_spmd(nc, in_maps, core_ids=list(range(NCORES)))
    _CACHE["last_results"] = res
    ln_total = 0.0
    pos_total = 0.0
    for r in res.results:
        ln_total += r["loss_rows"].astype(np.float64).sum()
        pos_total += r["pospart"].astype(np.float64).sum()
    loss = (ln_total - pos_total * POS_SCALE * INV_T) / N
    return np.float32(loss)


# revision 20
# speedup vs baseline: 1.4271x; 1.1249x over previous
"""InfoNCE loss kernel for Trainium2, 8 NeuronCores (v3).

Reference computation:
    z = l2_normalize(concat([polyline_embs, c_embs]))   # [8192, 512]
    sim = z @ z.T                                       # [8192, 8192]
    denom_i = sum_{j != i} exp(sim_ij / T)
    pos_i   = sim[i, i +- B]
    loss    = mean(log(denom_i) - pos_i / T)

Design (per core, identical SPMD program):
  - Host ships bf16 x in a column-tiled layout [n][p][c][col], ROTATED
    per core so the core's own 1024 rows are local column tiles 0-1
    (row sums are column-order invariant).
  - Norms WITHOUT the scalar engine: sq = x*x (DVE, all-bf16 fast
    path), ssq via ones-matmuls, then rsqrt by a Quake-style Newton
    iteration on DVE, done on a [128, W]-shaped copy of ssq obtained
    via a DRAM round-trip (the [1, N] shape would serialize on one
    DVE lane). rb = 128/||x||.
  - za8 = e4m3(x * rb) = e4m3(128 z): DVE multiply against the
    matmul-broadcast rb, fp8 output.
  - Main: 6 column groups (5x3 + 1x1 tiles) x 8 row tiles: fp8
    DoubleRow matmuls (K=256) into a [128, 3*512] PSUM group, one exp
    activation per group with accum_out row sums.
    exp(psum * 2^-13) = exp(sim/T).
  - The scalar engine runs ONLY the main exps and one final Ln: two
    activation-table loads total.
  - Epilogue: denom = rowsum - e^2 (self-term is constant 1 up to
    quantization, error ~1e-5 of the ~8200 denominator); one Ln.
    Positives need no log: sum_i pos_i is a single DVE multiply +
    reduce over za8 (mine tiles 0-1 x partner tiles 8-9).
  - Host: loss = (sum ln denom - (sum pospart)*2^-14/T) / 8192.
"""

import numpy as np
import ml_dtypes

B = 4096
D = 512
N = 2 * B            # 8192 rows of sim
NCORES = 8
RPC = N // NCORES    # 1024 rows per core
P = 128              # partitions
NT = 512             # column-tile width
NTILES = N // NT     # 16 column tiles
CT = D // P          # 4 contraction chunks of 128
ITILES = RPC // P    # 8 row tiles per core
GROUPS = [3, 3, 3, 3, 3, 1]          # n-tiles per psum group (sum 16)
NGRP = len(GROUPS)
# ssq scaled by 2^-14 before rsqrt -> rb = 128/||x||, za8 = e4m3(128 z)
SSQ_SCALE = 2.0 ** -14
# psum = za8 . za8 = 2^14 z.z ; exp(psum * EXP_SCALE) = exp(sim / T), T=0.5
EXP_SCALE = 2.0 ** -13
POS_SCALE = 2.0 ** -14
SELF_TERM = float(np.exp(2.0))   # exp(sim_ii / T), sim_ii = 1
INV_T = 2.0
MAGIC1 = 0x5F3759E0              # rsqrt magic + 1 (for ~x + magic + 1)

_CACHE = {}


def _build_bass():
    """Trace the per-core Bass program (identical for all 8 cores)."""
    import concourse.bass as bass
    import concourse.tile as tile
    from concourse import bacc, mybir

    dt = mybir.dt
    AF = mybir.ActivationFunctionType
    ALU = mybir.AluOpType
    DR = mybir.MatmulPerfMode.DoubleRow

    nc = bacc.Bacc(None, target_bir_lowering=False, debug=False, num_swdge_queues=4)

    xa_d = nc.dram_tensor("xa", [NTILES, P, CT, NT], dt.bfloat16,
                          kind="ExternalInput")
    out_d = nc.dram_tensor("loss_rows", [P, ITILES], dt.float32,
                           kind="ExternalOutput")
    pp_d = nc.dram_tensor("pospart", [P, 1], dt.float32, kind="ExternalOutput")
    dbg_d = nc.dram_tensor("dbg", [P, ITILES, 2], dt.float32,
                           kind="ExternalOutput")
    # DRAM bounce buffers for the [1, 2048] <-> [16, 128] reshape
    # (16 rows keeps DMA descriptors big; 128 partitions would mean 128
    # tiny descriptors and ~10us per bounce)
    batches = [(0, 4), (4, 8), (8, 12), (12, NTILES)]
    BP = 16                                   # bounce partitions
    scr_f = [nc.dram_tensor(f"scrf_{i}", [BP, (b1 - b0) * NT // BP], dt.float32,
                            kind="Internal") for i, (b0, b1) in enumerate(batches)]
    scr_b = [nc.dram_tensor(f"scrb_{i}", [BP, (b1 - b0) * NT // BP], dt.bfloat16,
                            kind="Internal") for i, (b0, b1) in enumerate(batches)]

    from contextlib import ExitStack

    with tile.TileContext(nc) as tc, ExitStack() as ctx:
        const = ctx.enter_context(tc.tile_pool(name="const", bufs=1))
        persist = ctx.enter_context(tc.tile_pool(name="persist", bufs=1))
        sqring = ctx.enter_context(tc.tile_pool(name="sqring", bufs=3))
        small = ctx.enter_context(tc.tile_pool(name="small", bufs=2))
        nwt = ctx.enter_context(tc.tile_pool(name="nwt", bufs=1))
        psum_pre = ctx.enter_context(tc.tile_pool(name="psum_pre", bufs=2,
                                                  space="PSUM"))
        psum_m = ctx.enter_context(tc.tile_pool(name="psum_m", bufs=2,
                                                space="PSUM"))

        ones_col = const.tile([P, 1], dt.bfloat16)
        nc.vector.memset(ones_col, 1.0)
        ones_row = const.tile([1, P], dt.bfloat16)
        nc.vector.memset(ones_row, 1.0)

        xa = persist.tile([P, NTILES, CT, NT], dt.bfloat16)
        za8 = persist.tile([P, NTILES, CT, NT], dt.float8e4)
        ssq_sb = persist.tile([1, N], dt.float32)
        rb_sb = persist.tile([1, N], dt.bfloat16)
        rowpart = persist.tile([P, ITILES, NGRP], dt.float32)

        # ---------------- input DMAs (all 16 tiles, pipelined) -----------
        # HWDGE via the idle sync engine: cheap triggers, and keeps the
        # gpsimd SWDGE queues free for the low-latency bounce DMAs.
        for n in range(NTILES):
            nc.sync.dma_start(out=xa[:, n, :, :], in_=xa_d[n])

        # ---------------- prologue helpers ------------------------------
        def ssq_tiles(tiles):
            """sq + ssq matmuls + scaled copy to ssq_sb for given tiles."""
            for n in tiles:
                sq = sqring.tile([P, CT, NT], dt.bfloat16,
                                 name=f"sq_{n}", tag="sq", bufs=3)
                nc.vector.tensor_mul(sq, xa[:, n, :, :], xa[:, n, :, :])
                ps = psum_pre.tile([P, NT], dt.float32,
                                   name=f"ssq_{n}", tag="pre")
                for c in range(CT):
                    nc.tensor.matmul(ps[0:1, :], ones_col, sq[:, c, :],
                                     start=(c == 0), stop=(c == CT - 1))
                nc.vector.tensor_scalar_mul(
                    ssq_sb[0:1, n * NT:(n + 1) * NT], ps[0:1, :], SSQ_SCALE)

        nwt_yt = {}

        def newton_fwd(bi):
            """Launch the ssq transpose bounce (latency hides under other
            DVE work emitted between fwd and compute)."""
            b0, b1 = batches[bi]
            lo, hi = b0 * NT, b1 * NT
            W = (hi - lo) // BP
            nc.gpsimd.dma_start(out=scr_f[bi][:], in_=ssq_sb[0:1, lo:hi])
            yt = nwt.tile([BP, W], dt.float32, name=f"yt_{bi}")
            nc.gpsimd.dma_start(out=yt, in_=scr_f[bi][:])
            nwt_yt[bi] = yt

        def newton_compute(bi):
            """rb = rsqrt(ssq*2^-14) on the [16, W] reshape."""
            b0, b1 = batches[bi]
            lo, hi = b0 * NT, b1 * NT
            W = (hi - lo) // BP
            yt = nwt_yt[bi]
            # y0 bits = MAGIC - (bits(y) >> 1), via float-domain value math
            # (DVE add is fp32 internally; the mod-2^32 trick would overflow)
            tu = nwt.tile([BP, W], dt.uint32, name=f"tu_{bi}")
            nc.vector.tensor_scalar(tu, yt.bitcast(dt.uint32), 1, None,
                                    op0=ALU.arith_shift_right)
            tf = nwt.tile([BP, W], dt.float32, name=f"tf_{bi}")
            nc.vector.tensor_copy(tf, tu)
            nc.vector.tensor_scalar(tf, tf, -1.0, float(MAGIC1 - 1),
                                    op0=ALU.mult, op1=ALU.add)
            y0u = nwt.tile([BP, W], dt.uint32, name=f"y0u_{bi}")
            nc.vector.tensor_copy(y0u, tf)
            h = nwt.tile([BP, W], dt.float32, name=f"h_{bi}")
            nc.vector.tensor_scalar_mul(h, yt, 0.5)
            y = y0u.bitcast(dt.float32)
            for it in range(2):
                t2 = nwt.tile([BP, W], dt.float32, name=f"t2_{bi}_{it}")
                nc.vector.tensor_mul(t2, y, y)
                nc.vector.tensor_mul(t2, t2, h)
                nc.vector.tensor_scalar(t2, t2, -1.0, 1.5,
                                        op0=ALU.mult, op1=ALU.add)
                yn = nwt.tile([BP, W], dt.float32, name=f"yn_{bi}_{it}")
                nc.vector.tensor_mul(yn, y, t2)
                y = yn
            rbT = nwt.tile([BP, W], dt.bfloat16, name=f"rbT_{bi}")
            nc.vector.tensor_copy(rbT, y)
            nc.gpsimd.dma_start(out=scr_b[bi][:], in_=rbT)
            nc.gpsimd.dma_start(out=rb_sb[0:1, lo:hi], in_=scr_b[bi][:])

        def za8_tiles(tiles):
            """broadcast rb + quantize za8 for given tiles."""
            for n in tiles:
                rbb = psum_pre.tile([P, NT], dt.float32,
                                    name=f"rbb_{n}", tag="pre")
                nc.tensor.matmul(rbb, ones_row,
                                 rb_sb[0:1, n * NT:(n + 1) * NT])
                rb_bc = bass.AP(tensor=rbb.tensor, offset=rbb.offset,
                                ap=[rbb.ap[0], [0, CT], rbb.ap[1]])
                nc.vector.tensor_mul(za8[:, n, :, :], xa[:, n, :, :], rb_bc)

        def main_group(g, n0, gsz):
            for i in range(ITILES):
                pm = psum_m.tile([P, 3, NT], dt.float32,
                                 name=f"pm_{g}_{i}", tag="pm")
                nm, mc = divmod(i, 4)
                for gg in range(2):
                    lhsT = za8[:, nm, 2 * gg:2 * gg + 2, mc * P:(mc + 1) * P]
                    for nn in range(gsz):
                        nc.tensor.matmul(pm[:, nn, :], lhsT,
                                         za8[:, n0 + nn, 2 * gg:2 * gg + 2, :],
                                         start=(gg == 0), stop=(gg == 1),
                                         perf_mode=DR)
                ej = sqring.tile([P, 3, NT], dt.bfloat16,
                                 name=f"ej_{g}_{i}", tag="ej", bufs=2)
                nc.scalar.activation(ej[:, 0:gsz, :], pm[:, 0:gsz, :], AF.Exp,
                                     scale=EXP_SCALE,
                                     accum_out=rowpart[:, i, g:g + 1])

        # ---------------- phase order ------------------------------------
        # All sq/ssq upfront (DMA-paced). Batch A's rb comes from the
        # scalar engine (idle before the main exps; avoids two ~5us DMA
        # bounce round-trips on the critical path). Batches B-D use the
        # DVE Newton whose bounce DMAs fly hidden under za8 work.
        gstart = np.cumsum([0] + GROUPS).tolist()

        for bi in range(4):
            ssq_tiles(range(*batches[bi]))
            if bi > 0:
                newton_fwd(bi)

        # batch A: rb = exp(-0.5 ln(ssq)) on the scalar engine
        lnA = nwt.tile([1, batches[0][1] * NT], dt.float32, name="lnA")
        nc.scalar.activation(lnA, ssq_sb[0:1, 0:batches[0][1] * NT], AF.Ln)
        nc.scalar.activation(rb_sb[0:1, 0:batches[0][1] * NT], lnA, AF.Exp,
                             scale=-0.5)
        za8_tiles(range(*batches[0]))
        main_group(0, gstart[0], GROUPS[0])     # tiles 0-2
        newton_compute(1)
        za8_tiles(range(*batches[1]))
        main_group(1, gstart[1], GROUPS[1])     # tiles 3-5
        newton_compute(2)
        za8_tiles(range(*batches[2]))
        main_group(2, gstart[2], GROUPS[2])     # tiles 6-8
        newton_compute(3)
        za8_tiles(range(*batches[3]))
        for g in range(3, NGRP):
            main_group(g, gstart[g], GROUPS[g])

        # ---------------- positives partial (no log needed) --------------
        junk = small.tile([P, 2, CT, NT], dt.bfloat16)
        nc.vector.tensor_mul(junk, za8[:, 0:2, :, :], za8[:, 8:10, :, :])
        jf = bass.AP(tensor=junk.tensor, offset=junk.offset,
                     ap=[junk.ap[0], [1, 2 * CT * NT]])
        pp = small.tile([P, 1], dt.float32)
        nc.vector.tensor_reduce(pp, jf, axis=mybir.AxisListType.X, op=ALU.add)
        nc.gpsimd.dma_start(out=pp_d[:], in_=pp)

        # ---------------- epilogue: per-row log denominators -------------
        rowsum = small.tile([P, ITILES], dt.float32)
        nc.vector.tensor_reduce(rowsum, rowpart, axis=mybir.AxisListType.X,
                                op=ALU.add)
        denom = small.tile([P, ITILES], dt.float32)
        nc.vector.tensor_scalar_add(denom, rowsum, -SELF_TERM)
        loss_t = small.tile([P, ITILES], dt.float32)
        nc.scalar.activation(loss_t, denom, AF.Ln)
        nc.gpsimd.dma_start(out=out_d[:], in_=loss_t)

        dbg = small.tile([P, ITILES, 2], dt.float32)
        nc.vector.tensor_copy(dbg[:, :, 0], rowsum)
        nc.vector.tensor_copy(dbg[:, :, 1], denom)
        nc.gpsimd.dma_start(out=dbg_d[:], in_=dbg)

    nc.compile()
    return nc


def _get_nc():
    if "nc" not in _CACHE:
        _CACHE["nc"] = _build_bass()
    return _CACHE["nc"]


def _prep_inputs(polyline_embs, c_embs):
    """Host-side shard/tile prep. Returns in_maps for the 8 cores."""
    bf16 = ml_dtypes.bfloat16
    x = np.concatenate([np.asarray(polyline_embs, np.float32),
                        np.asarray(c_embs, np.float32)], axis=0)  # [8192, 512]
    xb = x.astype(bf16)

    in_maps = []
    for k in range(NCORES):
        qk = np.roll(xb, -k * RPC, axis=0)                        # mine first
        xt = np.ascontiguousarray(qk.T)                           # [512, 8192]
        xa = np.ascontiguousarray(
            xt.reshape(CT, P, NTILES, NT).transpose(2, 1, 0, 3))  # [16,128,4,512]
        in_maps.append({"xa": xa})
    return in_maps


def kernel(polyline_embs, c_embs):
    from concourse.bass_utils import run_bass_kernel_spmd

    nc = _get_nc()
    in_maps = _prep_inputs(polyline_embs, c_embs)
    res = run_bass_kernel_spmd(nc, in_maps, core_ids=list(range(NCORES)))
    _CACHE["last_results"] = res
    ln_total = 0.0
    pos_total = 0.0
    for r in res.results:
        ln_total += r["loss_rows"].astype(np.float64).sum()
        pos_total += r["pospart"].astype(np.float64).sum()
    loss = (ln_total - pos_total * POS_SCALE * INV_T) / N
    return np.float32(loss)
